# revision 29
# baseline (speedup 1.0000x reference)
"""Bass/Tile kernel for nn_Attention_49959059587521 on 8 TRN2 NeuronCores.

Math per (batch b, head h), with Q,K,V,Q2,K2 = [2048, 64] slices:
    S    = (Q @ K^T) * 0.125                    # [2048, 2048]
    P    = softmax(S, axis=-1)
    gate = sigmoid((Q2 @ sum_n(K2)) * 0.125)    # [2048]
    out  = (P * gate[:, None]) @ V              # [2048, 64]

Sharding: 32 (b, h) pairs over 8 cores -> core i handles b = i//2 and the 4
heads h in [4*(i%2), 4*(i%2)+4), i.e. the channel slice [256*(i%2), +256).
No cross-core communication.

Per-core algorithm (fully on device):
  - S^T[k, q] = K^T(stationary) x Q^T(moving) via bf16 matmuls (1 cyc/row).
  - exp fused on ScalarE reading PSUM directly (scale=0.125 via free affine);
    no max-subtraction needed: logits are ~N(0,1), |S| < ~7, exp is safe in f32.
  - O^T = V'^T @ E accumulated in PSUM over the 16 k-tiles, where V' = [V; ones]
    so row 64 of O^T is the softmax denominator (free rowsum).
  - gate computed as 1/(1+exp(-z*scale)) (exp table only; inf-safe).
  - O^T 128-column blocks PE-transposed back to [q, d]; the PSUM->SBUF copy
    fuses the (gate * 1/rowsum) per-row scale on the VectorE.
"""

import functools
from contextlib import ExitStack

import numpy as np

import concourse.bass as bass
import concourse.mybir as mybir
import concourse.tile as tile
from concourse import bacc, bass_isa, bass_utils
from concourse.masks import make_identity

F32 = mybir.dt.float32
F32R = mybir.dt.float32r

B, NT, C, H = 4, 2048, 512, 8
HD = 64
SCALE = HD ** -0.5  # 0.125
P = 128
NO = NT // P            # 16 n-tiles
NH4 = 4                 # heads per core
CW = NH4 * HD           # 256 channels per core
NHALF = 2               # q processed in two halves of 1024
QH = NT // NHALF        # 1024
BF16 = mybir.dt.bfloat16
MM_DT = BF16            # dtype of matmul operands (qT/kT/V'/E)


def _build(use_sigmoid: bool):
    nc = bacc.Bacc("TRN2", target_bir_lowering=False)
    q_d = nc.dram_tensor("q", [NT, CW], F32, kind="ExternalInput")
    k_d = nc.dram_tensor("k", [NT, CW], F32, kind="ExternalInput")
    v_d = nc.dram_tensor("v", [NT, CW], F32, kind="ExternalInput")
    if use_sigmoid:
        q2_d = nc.dram_tensor("q2", [NT, CW], F32, kind="ExternalInput")
        k2_d = nc.dram_tensor("k2", [NT, CW], F32, kind="ExternalInput")
    out_d = nc.dram_tensor("out", [NT, CW], F32, kind="ExternalOutput")

    with tile.TileContext(nc) as tc, ExitStack() as ctx:
        singles = ctx.enter_context(tc.tile_pool(name="singles", bufs=1))
        tpool = ctx.enter_context(tc.tile_pool(name="tp", bufs=2))
        epool = ctx.enter_context(tc.tile_pool(name="ep", bufs=3))
        opool = ctx.enter_context(tc.tile_pool(name="op", bufs=2))
        # PSUM: st 2x[128,1024] = 4 banks, acc 1x[65,1024] = 2 banks,
        # tr 2x[<=128,<=512] = 2 banks. Total 8 banks.
        ps_st = ctx.enter_context(tc.tile_pool(name="ps_st", bufs=2, space="PSUM"))
        ps_ac = ctx.enter_context(tc.tile_pool(name="ps_ac", bufs=1, space="PSUM"))
        ps_tr = ctx.enter_context(tc.tile_pool(name="ps_tr", bufs=2, space="PSUM"))

        def tr_tile(shape, dtype=F32):
            return ps_tr.tile(shape, dtype, tag="ptr", name="ptr")

        # ---- stage A: bulk input loads ([n, c] -> [p, o, c] tiling) ----
        # All input loads ride the Pool SW-DGE queue: it issues ~1us per DMA
        # and spreads transfers over the 8 HW DMA engines (~240+ GB/s),
        # while the SP HW-DGE executes DIRECT2D synchronously at ~80 GB/s.
        # Emission order == consumption order: q, k, v(first half), q2, ...
        def load_tiled(dram, nm, split=True):
            t = singles.tile([P, NO, CW], F32, name=nm, tag=nm)
            src = dram.ap().rearrange("(o p) c -> p o c", p=P)
            if split:
                for g in range(4):
                    nc.gpsimd.dma_start(
                        t[:, 4 * g : 4 * (g + 1), :], src[:, 4 * g : 4 * (g + 1), :]
                    )
            else:
                nc.gpsimd.dma_start(t, src)
            return t

        q_sb = singles.tile([P, NO, CW], F32, name="q_sb", tag="q_sb")
        k_sb = singles.tile([P, NO, CW], F32, name="k_sb", tag="k_sb")
        q_src = q_d.ap().rearrange("(o p) c -> p o c", p=P)
        k_src = k_d.ap().rearrange("(o p) c -> p o c", p=P)
        for g in range(8):
            sl = slice(2 * g, 2 * (g + 1))
            nc.gpsimd.dma_start(q_sb[:, sl, :], q_src[:, sl, :])
            nc.gpsimd.dma_start(k_sb[:, sl, :], k_src[:, sl, :])

        # identities (gpsimd; emitted after the q/k DMA triggers, ready ~5us)
        ident = singles.tile([P, P], F32)
        make_identity(nc, ident)
        ident_b = singles.tile([P, P], BF16)
        make_identity(nc, ident_b)

        # zero halves of the pair-0 kT weight tiles (gpsimd, off the DVE path)
        kTz_all = []
        for jp in range(NH4 // 2):
            kTza = tpool.tile([P, NT], MM_DT, tag="kTza", name="kTza")
            kTzb = tpool.tile([P, NT], MM_DT, tag="kTzb", name="kTzb")
            kTz_all.extend([kTza, kTzb])

        # V loaded contiguously (1KB runs; the per-head strided load was
        # descriptor-bound at ~8-16us). V' = [V | ones] built on-chip.
        v_sb = load_tiled(v_d, "v_sb")
        v1r = singles.tile([P, NO, NH4, HD + 1], MM_DT)
        if use_sigmoid:
            k2_sb = load_tiled(k2_d, "k2_sb", split=False)

        # zero halves of the kT weight tiles on DVE: fills the DVE idle
        # window while the q/k DMAs land, ahead of the casts (uint32 view
        # halves the element count -> 2x faster memset)
        U32 = mybir.dt.uint32
        nc.vector.memset(kTz_all[0][HD:P, :].bitcast(U32), 0)
        nc.vector.memset(kTz_all[1][0:HD, :].bitcast(U32), 0)
        nc.vector.memset(kTz_all[2][HD:P, :].bitcast(U32), 0)
        nc.vector.memset(kTz_all[3][0:HD, :].bitcast(U32), 0)

        # bf16 casts of q/k for the logit matmuls, per o-group, interleaved
        # q/k to match DMA arrival order
        qbf = singles.tile([P, NO, CW], BF16)
        kbf = singles.tile([P, NO, CW], BF16)
        for g in range(4):
            for src, dst in ((q_sb, qbf), (k_sb, kbf)):
                nc.vector.tensor_copy(
                    dst[:, 4 * g : 4 * (g + 1), :], src[:, 4 * g : 4 * (g + 1), :]
                )

        if use_sigmoid:
            q2_sb = load_tiled(q2_d, "q2_sb", split=False)
            k2o = singles.tile([P, CW], F32)
            k2b_sb = singles.tile([P, CW], F32)

        out_ap3 = out_d.ap().rearrange("(o p) c -> p o c", p=P)

        # ---- stage B: stacked transposes: qT2 [128, 2048] covers two heads
        # (head 2jp on partitions 0:64, 2jp+1 on 64:128); kT is split into
        # two zero-padded weight tensors so the logit matmuls contract over
        # the full 128 partitions (K=128 runs at 1 cyc/col; K=64 would run
        # at 2 and K-switches cost more). Pair 1's transposes are emitted
        # between head 0 and head 1 so they hide under the exp stream.
        qT2s = [None, None]

        def stage_b(jp):
            cp = 2 * HD * jp
            qT2 = tpool.tile([P, NT], MM_DT, tag="qT2", name="qT2")
            qT2s[jp] = qT2
            kTza, kTzb = kTz_all[2 * jp], kTz_all[2 * jp + 1]
            for g in range(NO // 4):
                tp = tr_tile([P, 4 * P], BF16)
                for u in range(4):
                    o = 4 * g + u
                    nc.tensor.transpose(
                        tp[:, P * u : P * (u + 1)],
                        qbf[:, o, cp : cp + P],
                        ident_b,
                    )
                nc.vector.tensor_copy(qT2[:, 4 * P * g : 4 * P * (g + 1)], tp)
                tp2 = tr_tile([P, 4 * P], BF16)
                for u in range(4):
                    o = 4 * g + u
                    nc.tensor.transpose(
                        tp2[:, P * u : P * (u + 1)],
                        kbf[:, o, cp : cp + P],
                        ident_b,
                    )
                nc.vector.tensor_copy(
                    kTza[0:HD, 4 * P * g : 4 * P * (g + 1)], tp2[0:HD]
                )
                nc.vector.tensor_copy(
                    kTzb[HD:P, 4 * P * g : 4 * P * (g + 1)], tp2[HD:P]
                )

        stage_b(0)
        nc.vector.tensor_copy(v1r[:, :, 0, 0:HD], v_sb[:, :, 0:HD])
        nc.gpsimd.memset(v1r[:, :, 0, HD : HD + 1], 1.0)

        for j in range(NH4):  # local head
            jp, jj = divmod(j, 2)
            ch = HD * j
            qT2 = qT2s[jp]
            kTz = kTz_all[2 * jp + jj]
            if j > 0:
                nc.vector.tensor_copy(v1r[:, :, j, 0:HD], v_sb[:, :, ch : ch + HD])
                nc.gpsimd.memset(v1r[:, :, j, HD : HD + 1], 1.0)

            obuf = opool.tile([P, NO, HD], F32, tag="obuf")
            gte = None
            for h in range(NHALF):  # q half
                q0 = QH * h
                # ---- stage D: S^T -> exp -> O^T accumulation ----
                acc = ps_ac.tile([HD + 1, QH], F32, tag="pac")
                for t in range(NO):
                    st = ps_st.tile([P, QH], F32, tag="pst")
                    for s2 in range(QH // 512):
                        nc.tensor.matmul(
                            st[:, 512 * s2 : 512 * (s2 + 1)],
                            kTz[:, P * t : P * (t + 1)],
                            qT2[:, q0 + 512 * s2 : q0 + 512 * (s2 + 1)],
                            start=True,
                            stop=True,
                        )
                    et = epool.tile([P, QH], MM_DT, tag="et")
                    nc.scalar.activation(
                        et, st, mybir.ActivationFunctionType.Exp, scale=SCALE
                    )
                    for s2 in range(QH // 512):
                        nc.tensor.matmul(
                            acc[:, 512 * s2 : 512 * (s2 + 1)],
                            v1r[:, t, j, :],
                            et[:, 512 * s2 : 512 * (s2 + 1)],
                            start=(t == 0),
                            stop=(t == NO - 1),
                        )
                if use_sigmoid and h == 0:
                    # gate = sigmoid(scale * q2 . k2sum) as [128, 16]; emitted
                    # after half-0's exps so the late q2/k2 never stall the
                    # ScalarE exp stream, but ready before phase 2 needs it
                    if j == 0:
                        for cc in range(4):
                            csl = slice(HD * cc, HD * (cc + 1))
                            nc.vector.reduce_sum(
                                out=k2o[:, csl],
                                in_=k2_sb[:, :, csl].rearrange("p o c -> p c o"),
                                axis=mybir.AxisListType.X,
                            )
                        nc.gpsimd.partition_all_reduce(
                            k2b_sb, k2o, channels=P, reduce_op=bass_isa.ReduceOp.add
                        )
                    zt = opool.tile([P, NO, HD], F32, tag="zt")
                    nc.vector.tensor_mul(
                        zt,
                        q2_sb[:, :, ch : ch + HD],
                        k2b_sb[:, None, ch : ch + HD].to_broadcast((P, NO, HD)),
                    )
                    z = opool.tile([P, NO], F32, tag="z")
                    nc.vector.reduce_sum(out=z, in_=zt, axis=mybir.AxisListType.X)
                    eg = opool.tile([P, NO], F32, tag="eg")
                    nc.scalar.activation(
                        eg, z, mybir.ActivationFunctionType.Exp, scale=-SCALE
                    )
                    nc.vector.tensor_scalar_add(eg, eg, 1.0)
                    gte = opool.tile([P, NO], F32, tag="gte")
                    nc.vector.reciprocal(gte, eg)

                ot_sb = opool.tile([HD + 1, QH], F32, tag="ot", bufs=3, name="ot_sb")
                nc.vector.tensor_copy(ot_sb, acc)

                # ---- phase 2 (inline per half): transpose + normalize ----
                for u in range(QH // P):
                    i = (QH // P) * h + u
                    tr = tr_tile([P, HD + 1])
                    nc.tensor.transpose(
                        tr,
                        ot_sb[:, P * u : P * (u + 1)],
                        ident[: HD + 1, : HD + 1],
                    )
                    rcp = opool.tile([P, 1], F32, tag="rcp")
                    nc.vector.reciprocal(rcp, tr[:, HD : HD + 1])
                    if use_sigmoid:
                        fac = opool.tile([P, 1], F32, tag="fac")
                        nc.vector.tensor_mul(fac, rcp, gte[:, i : i + 1])
                    else:
                        fac = rcp
                    nc.vector.tensor_scalar_mul(obuf[:, i, :], tr[:, 0:HD], fac)

                nc.sync.dma_start(
                    out_ap3[:, 8 * h : 8 * (h + 1), ch : ch + HD],
                    obuf[:, 8 * h : 8 * (h + 1), :],
                )

            if j == 0:
                stage_b(1)  # pair-1 transposes hide under head 0/1 compute

    nc.compile()
    return nc


@functools.lru_cache(maxsize=2)
def _graph(use_sigmoid: bool):
    return _build(use_sigmoid)


def _shard(a: np.ndarray, i: int) -> np.ndarray:
    b, hg = divmod(i, 2)
    return np.ascontiguousarray(a[b, :, hg * CW : (hg + 1) * CW], dtype=np.float32)


def run(inputs, trace: bool = False):
    use_sigmoid = bool(np.asarray(inputs["use_sigmoid"]).item())
    nc = _graph(use_sigmoid)
    in_maps = []
    for i in range(8):
        m = {
            "q": _shard(np.asarray(inputs["query"]), i),
            "k": _shard(np.asarray(inputs["key"]), i),
            "v": _shard(np.asarray(inputs["value"]), i),
        }
        if use_sigmoid:
            m["q2"] = _shard(np.asarray(inputs["query2"]), i)
            m["k2"] = _shard(np.asarray(inputs["key2"]), i)
        in_maps.append(m)
    res = bass_utils.run_bass_kernel_spmd(
        nc, in_maps, core_ids=list(range(8)), trace=trace
    )
    out = np.empty((B, NT, C), dtype=np.float32)
    for i in range(8):
        b, hg = divmod(i, 2)
        out[b, :, hg * CW : (hg + 1) * CW] = res.results[i]["out"]
    return out, res


def kernel(**inputs) -> np.ndarray:
    out, _ = run(inputs)
    return out


if __name__ == "__main__":
    rng = np.random.default_rng(0)
    fake = {
        "query": rng.standard_normal((B, NT, C), dtype=np.float32),
        "key": rng.standard_normal((B, NT, C), dtype=np.float32),
        "value": rng.standard_normal((B, NT, C), dtype=np.float32),
        "query2": rng.standard_normal((B, NT, C), dtype=np.float32),
        "key2": rng.standard_normal((B, NT, C), dtype=np.float32),
        "use_sigmoid": 1,
    }
    out = kernel(**fake)
    print("ran ok", out.shape, out.dtype)


# revision 32
# speedup vs baseline: 1.0537x; 1.0537x over previous
"""Bass/Tile kernel for nn_Attention_49959059587521 on 8 TRN2 NeuronCores.

Math per (batch b, head h), with Q,K,V,Q2,K2 = [2048, 64] slices:
    S    = (Q @ K^T) * 0.125                    # [2048, 2048]
    P    = softmax(S, axis=-1)
    gate = sigmoid((Q2 @ sum_n(K2)) * 0.125)    # [2048]
    out  = (P * gate[:, None]) @ V              # [2048, 64]

Sharding: 32 (b, h) pairs over 8 cores -> core i handles b = i//2 and the 4
heads h in [4*(i%2), 4*(i%2)+4), i.e. the channel slice [256*(i%2), +256).
No cross-core communication.

Per-core algorithm (fully on device):
  - S^T[k, q] = K^T(stationary) x Q^T(moving) via bf16 matmuls (1 cyc/row).
  - exp fused on ScalarE reading PSUM directly (scale=0.125 via free affine);
    no max-subtraction needed: logits are ~N(0,1), |S| < ~7, exp is safe in f32.
  - O^T = V'^T @ E accumulated in PSUM over the 16 k-tiles, where V' = [V; ones]
    so row 64 of O^T is the softmax denominator (free rowsum).
  - gate computed as 1/(1+exp(-z*scale)) (exp table only; inf-safe).
  - O^T 128-column blocks PE-transposed back to [q, d]; the PSUM->SBUF copy
    fuses the (gate * 1/rowsum) per-row scale on the VectorE.
"""

import functools
from contextlib import ExitStack

import numpy as np

import concourse.bass as bass
import concourse.mybir as mybir
import concourse.tile as tile
from concourse import bacc, bass_isa, bass_utils
from concourse.masks import make_identity

F32 = mybir.dt.float32
F32R = mybir.dt.float32r

B, NT, C, H = 4, 2048, 512, 8
HD = 64
SCALE = HD ** -0.5  # 0.125
P = 128
NO = NT // P            # 16 n-tiles
NH4 = 4                 # heads per core
CW = NH4 * HD           # 256 channels per core
NHALF = 2               # q processed in two halves of 1024
QH = NT // NHALF        # 1024
BF16 = mybir.dt.bfloat16
MM_DT = BF16            # dtype of matmul operands (qT/kT/V'/E)


def _build(use_sigmoid: bool):
    nc = bacc.Bacc("TRN2", target_bir_lowering=False)
    q_d = nc.dram_tensor("q", [NT, CW], F32, kind="ExternalInput")
    k_d = nc.dram_tensor("k", [NT, CW], F32, kind="ExternalInput")
    v_d = nc.dram_tensor("v", [NT, CW], F32, kind="ExternalInput")
    if use_sigmoid:
        q2_d = nc.dram_tensor("q2", [NT, CW], F32, kind="ExternalInput")
        k2_d = nc.dram_tensor("k2", [NT, CW], F32, kind="ExternalInput")
    out_d = nc.dram_tensor("out", [NT, CW], F32, kind="ExternalOutput")

    with tile.TileContext(nc) as tc, ExitStack() as ctx:
        singles = ctx.enter_context(tc.tile_pool(name="singles", bufs=1))
        tpool = ctx.enter_context(tc.tile_pool(name="tp", bufs=2))
        epool = ctx.enter_context(tc.tile_pool(name="ep", bufs=3))
        opool = ctx.enter_context(tc.tile_pool(name="op", bufs=2))
        # PSUM: st 2x[128,1024] = 4 banks, acc 1x[65,1024] = 2 banks,
        # tr 2x[<=128,<=512] = 2 banks. Total 8 banks.
        ps_st = ctx.enter_context(tc.tile_pool(name="ps_st", bufs=2, space="PSUM"))
        ps_ac = ctx.enter_context(tc.tile_pool(name="ps_ac", bufs=1, space="PSUM"))
        ps_tr = ctx.enter_context(tc.tile_pool(name="ps_tr", bufs=2, space="PSUM"))

        def tr_tile(shape, dtype=F32):
            return ps_tr.tile(shape, dtype, tag="ptr", name="ptr")

        # ---- stage A: bulk input loads ([n, c] -> [p, o, c] tiling) ----
        # All input loads ride the Pool SW-DGE queue: it issues ~1us per DMA
        # and spreads transfers over the 8 HW DMA engines (~240+ GB/s),
        # while the SP HW-DGE executes DIRECT2D synchronously at ~80 GB/s.
        # Emission order == consumption order: q, k, v(first half), q2, ...
        def load_tiled(dram, nm, split=True):
            t = singles.tile([P, NO, CW], F32, name=nm, tag=nm)
            src = dram.ap().rearrange("(o p) c -> p o c", p=P)
            if split:
                for g in range(4):
                    nc.gpsimd.dma_start(
                        t[:, 4 * g : 4 * (g + 1), :], src[:, 4 * g : 4 * (g + 1), :]
                    )
            else:
                nc.gpsimd.dma_start(t, src)
            return t

        q_sb = singles.tile([P, NO, CW], F32, name="q_sb", tag="q_sb")
        k_sb = singles.tile([P, NO, CW], F32, name="k_sb", tag="k_sb")
        q_src = q_d.ap().rearrange("(o p) c -> p o c", p=P)
        k_src = k_d.ap().rearrange("(o p) c -> p o c", p=P)
        for g in range(4):
            sl = slice(4 * g, 4 * (g + 1))
            nc.gpsimd.dma_start(q_sb[:, sl, :], q_src[:, sl, :])
            nc.gpsimd.dma_start(k_sb[:, sl, :], k_src[:, sl, :])

        # identities (gpsimd; emitted after the q/k DMA triggers, ready ~5us)
        ident = singles.tile([P, P], F32)
        make_identity(nc, ident)
        ident_b = singles.tile([P, P], BF16)
        make_identity(nc, ident_b)

        # zero halves of the pair-0 kT weight tiles (gpsimd, off the DVE path)
        kTz_all = []
        for jp in range(NH4 // 2):
            kTza = tpool.tile([P, NT], MM_DT, tag="kTza", name="kTza")
            kTzb = tpool.tile([P, NT], MM_DT, tag="kTzb", name="kTzb")
            kTz_all.extend([kTza, kTzb])

        # V loaded contiguously (1KB runs; the per-head strided load was
        # descriptor-bound at ~8-16us). V' = [V | ones] built on-chip.
        v_sb = load_tiled(v_d, "v_sb")
        v1r = singles.tile([P, NO, NH4, HD + 1], MM_DT)

        # zero halves of the kT weight tiles on DVE: fills the DVE idle
        # window while the q/k DMAs land, ahead of the casts (uint32 view
        # halves the element count -> 2x faster memset)
        U32 = mybir.dt.uint32
        nc.vector.memset(kTz_all[0][HD:P, :].bitcast(U32), 0)
        nc.vector.memset(kTz_all[1][0:HD, :].bitcast(U32), 0)
        nc.vector.memset(kTz_all[2][HD:P, :].bitcast(U32), 0)
        nc.vector.memset(kTz_all[3][0:HD, :].bitcast(U32), 0)

        # bf16 casts of q/k for the logit matmuls, per o-group, interleaved
        # q/k to match DMA arrival order
        qbf = singles.tile([P, NO, CW], BF16)
        kbf = singles.tile([P, NO, CW], BF16)
        for g in range(4):
            for src, dst in ((q_sb, qbf), (k_sb, kbf)):
                nc.vector.tensor_copy(
                    dst[:, 4 * g : 4 * (g + 1), :], src[:, 4 * g : 4 * (g + 1), :]
                )

        if use_sigmoid:
            q2_sb = load_tiled(q2_d, "q2_sb", split=False)
            # k2 rides the otherwise-idle SP queue in the background
            k2_sb = singles.tile([P, NO, CW], F32)
            nc.sync.dma_start(k2_sb, k2_d.ap().rearrange("(o p) c -> p o c", p=P))
            k2o = singles.tile([P, CW], F32)
            k2b_sb = singles.tile([P, CW], F32)

        out_ap3 = out_d.ap().rearrange("(o p) c -> p o c", p=P)

        # ---- stage B: stacked transposes: qT2 [128, 2048] covers two heads
        # (head 2jp on partitions 0:64, 2jp+1 on 64:128); kT is split into
        # two zero-padded weight tensors so the logit matmuls contract over
        # the full 128 partitions (K=128 runs at 1 cyc/col; K=64 would run
        # at 2 and K-switches cost more). Pair 1's transposes are emitted
        # between head 0 and head 1 so they hide under the exp stream.
        qT2s = [None, None]

        def stage_b(jp):
            cp = 2 * HD * jp
            qT2 = tpool.tile([P, NT], MM_DT, tag="qT2", name="qT2")
            qT2s[jp] = qT2
            kTza, kTzb = kTz_all[2 * jp], kTz_all[2 * jp + 1]
            for g in range(NO // 4):
                tp = tr_tile([P, 4 * P], BF16)
                for u in range(4):
                    o = 4 * g + u
                    nc.tensor.transpose(
                        tp[:, P * u : P * (u + 1)],
                        qbf[:, o, cp : cp + P],
                        ident_b,
                    )
                nc.vector.tensor_copy(qT2[:, 4 * P * g : 4 * P * (g + 1)], tp)
                tp2 = tr_tile([P, 4 * P], BF16)
                for u in range(4):
                    o = 4 * g + u
                    nc.tensor.transpose(
                        tp2[:, P * u : P * (u + 1)],
                        kbf[:, o, cp : cp + P],
                        ident_b,
                    )
                nc.vector.tensor_copy(
                    kTza[0:HD, 4 * P * g : 4 * P * (g + 1)], tp2[0:HD]
                )
                nc.vector.tensor_copy(
                    kTzb[HD:P, 4 * P * g : 4 * P * (g + 1)], tp2[HD:P]
                )

        stage_b(0)
        nc.vector.tensor_copy(v1r[:, :, 0, 0:HD], v_sb[:, :, 0:HD])
        nc.gpsimd.memset(v1r[:, :, 0, HD : HD + 1], 1.0)
        if use_sigmoid:
            for cc in range(4):
                csl = slice(HD * cc, HD * (cc + 1))
                nc.vector.reduce_sum(
                    out=k2o[:, csl],
                    in_=k2_sb[:, :, csl].rearrange("p o c -> p c o"),
                    axis=mybir.AxisListType.X,
                )
            nc.gpsimd.partition_all_reduce(
                k2b_sb, k2o, channels=P, reduce_op=bass_isa.ReduceOp.add
            )

        for j in range(NH4):  # local head
            jp, jj = divmod(j, 2)
            ch = HD * j
            qT2 = qT2s[jp]
            kTz = kTz_all[2 * jp + jj]
            if j > 0:
                nc.vector.tensor_copy(v1r[:, :, j, 0:HD], v_sb[:, :, ch : ch + HD])
                nc.gpsimd.memset(v1r[:, :, j, HD : HD + 1], 1.0)

            obuf = opool.tile([P, NO, HD], F32, tag="obuf")
            gte = None
            ots = []
            for h in range(NHALF):  # q half
                q0 = QH * h
                # ---- stage D: S^T -> exp -> O^T accumulation ----
                acc = ps_ac.tile([HD + 1, QH], F32, tag="pac")
                for t in range(NO):
                    st = ps_st.tile([P, QH], F32, tag="pst")
                    for s2 in range(QH // 512):
                        nc.tensor.matmul(
                            st[:, 512 * s2 : 512 * (s2 + 1)],
                            kTz[:, P * t : P * (t + 1)],
                            qT2[:, q0 + 512 * s2 : q0 + 512 * (s2 + 1)],
                            start=True,
                            stop=True,
                        )
                    et = epool.tile([P, QH], MM_DT, tag="et")
                    nc.scalar.activation(
                        et, st, mybir.ActivationFunctionType.Exp, scale=SCALE
                    )
                    for s2 in range(QH // 512):
                        nc.tensor.matmul(
                            acc[:, 512 * s2 : 512 * (s2 + 1)],
                            v1r[:, t, j, :],
                            et[:, 512 * s2 : 512 * (s2 + 1)],
                            start=(t == 0),
                            stop=(t == NO - 1),
                        )
                ot_sb = opool.tile([HD + 1, QH], F32, tag="ot", bufs=3, name="ot_sb")
                nc.vector.tensor_copy(ot_sb, acc)
                ots.append(ot_sb)

            # gate = sigmoid(scale * q2 . k2sum) as [128, 16]; emitted after
            # both halves' exps so the late q2/k2 never stall the exp stream
            if use_sigmoid:
                zt = opool.tile([P, NO, HD], F32, tag="zt")
                nc.vector.tensor_mul(
                    zt,
                    q2_sb[:, :, ch : ch + HD],
                    k2b_sb[:, None, ch : ch + HD].to_broadcast((P, NO, HD)),
                )
                z = opool.tile([P, NO], F32, tag="z")
                nc.vector.reduce_sum(out=z, in_=zt, axis=mybir.AxisListType.X)
                eg = opool.tile([P, NO], F32, tag="eg")
                nc.scalar.activation(
                    eg, z, mybir.ActivationFunctionType.Exp, scale=-SCALE
                )
                nc.vector.tensor_scalar_add(eg, eg, 1.0)
                gte = opool.tile([P, NO], F32, tag="gte")
                nc.vector.reciprocal(gte, eg)

            # ---- phase 2: transpose + normalize (+gate) ----
            for h in range(NHALF):
                for u in range(QH // P):
                    i = (QH // P) * h + u
                    tr = tr_tile([P, HD + 1])
                    nc.tensor.transpose(
                        tr,
                        ots[h][:, P * u : P * (u + 1)],
                        ident[: HD + 1, : HD + 1],
                    )
                    rcp = opool.tile([P, 1], F32, tag="rcp")
                    nc.vector.reciprocal(rcp, tr[:, HD : HD + 1])
                    if use_sigmoid:
                        fac = opool.tile([P, 1], F32, tag="fac")
                        nc.vector.tensor_mul(fac, rcp, gte[:, i : i + 1])
                    else:
                        fac = rcp
                    nc.vector.tensor_scalar_mul(obuf[:, i, :], tr[:, 0:HD], fac)
                nc.sync.dma_start(
                    out_ap3[:, 8 * h : 8 * (h + 1), ch : ch + HD],
                    obuf[:, 8 * h : 8 * (h + 1), :],
                )

            if j == 0:
                stage_b(1)  # pair-1 transposes hide under head 0/1 compute

    nc.compile()
    return nc


@functools.lru_cache(maxsize=2)
def _graph(use_sigmoid: bool):
    return _build(use_sigmoid)


def _shard(a: np.ndarray, i: int) -> np.ndarray:
    b, hg = divmod(i, 2)
    return np.ascontiguousarray(a[b, :, hg * CW : (hg + 1) * CW], dtype=np.float32)


def run(inputs, trace: bool = False):
    use_sigmoid = bool(np.asarray(inputs["use_sigmoid"]).item())
    nc = _graph(use_sigmoid)
    in_maps = []
    for i in range(8):
        m = {
            "q": _shard(np.asarray(inputs["query"]), i),
            "k": _shard(np.asarray(inputs["key"]), i),
            "v": _shard(np.asarray(inputs["value"]), i),
        }
        if use_sigmoid:
            m["q2"] = _shard(np.asarray(inputs["query2"]), i)
            m["k2"] = _shard(np.asarray(inputs["key2"]), i)
        in_maps.append(m)
    res = bass_utils.run_bass_kernel_spmd(
        nc, in_maps, core_ids=list(range(8)), trace=trace
    )
    out = np.empty((B, NT, C), dtype=np.float32)
    for i in range(8):
        b, hg = divmod(i, 2)
        out[b, :, hg * CW : (hg + 1) * CW] = res.results[i]["out"]
    return out, res


def kernel(**inputs) -> np.ndarray:
    out, _ = run(inputs)
    return out


if __name__ == "__main__":
    rng = np.random.default_rng(0)
    fake = {
        "query": rng.standard_normal((B, NT, C), dtype=np.float32),
        "key": rng.standard_normal((B, NT, C), dtype=np.float32),
        "value": rng.standard_normal((B, NT, C), dtype=np.float32),
        "query2": rng.standard_normal((B, NT, C), dtype=np.float32),
        "key2": rng.standard_normal((B, NT, C), dtype=np.float32),
        "use_sigmoid": 1,
    }
    out = kernel(**fake)
    print("ran ok", out.shape, out.dtype)
